# revision 29
# baseline (speedup 1.0000x reference)
"""MultiHeadSemGConv Trainium2 kernel.

Computes, for x:[B,N,CIN], W:[H,2,CIN,HC], e:[H,N*K], bias:[H,HC],
rows/cols:[N*K] (int32 edge list):

    h = einsum('bnc,hscd->shbnd', x, W)             # two projections per head
    A = softmax(scatter(e at (rows,cols), NEG))     # [H,N,N]
    out[h,b] = diag(A)*h0 + (A - diag)@h1 + bias    # -> [B,N,H*HC]

Strategy: pure data-parallel over batch across 8 NeuronCores.  The tiny
[H,98,98] adjacency softmax is precomputed on host; the heavy lifting
(x projection + graph mixing over 100MB of activations) runs on device:

  per core (128 samples; x pre-cast to fp16 AND pre-transposed to
  [2, 128, 12544] host-side, so the device never transposes):
    - x^T chunk tiles [c(2x128), 1600] fp16 (16 samples + 32-col
      overlap) loaded by plain DMAs with contiguous 3.2KB descriptors.
    - phase 1, per sample b: h[128,512] = xT[:, 98b:98b+128].T @ Wall
      (2 accumulating fp16 matmuls, f32 PSUM), 2 samples per PSUM tile;
      rows [:98] copied (Act/DVE) into one of two persistent h tiles
      whose row 98 holds the bias pattern.  The 30-row overlap between
      consecutive samples keeps every lhsT at m=128.
    - phase 2, per 8-sample group, per head: ONE matmul with the
      host-built A_off^T (contract k=99: 98 nodes + bias row), then a
      fused DVE op  out = dg (.) h0 + psum  adds the diagonal part.
      Phase 2 of group g is interleaved into phase 1 of group g+1.
    - DMA out fp16 in node-major layout [98, 128, 256] (4KB
      descriptors); host transposes back to [B,N,256] f32.
"""

import os
import sys

import numpy as np

try:
    import concourse.bass as bass  # noqa: F401
except Exception:  # pragma: no cover - fresh grading dir fallback
    for p in ("/opt/trn_rl_repo", "/root/.axon_site/_ro/trn_rl_repo"):
        if os.path.isdir(p) and p not in sys.path:
            sys.path.insert(0, p)
    import concourse.bass as bass  # noqa: F401

# ---------------------------------------------------------------- constants
NLM = 98          # landmarks (graph nodes)
HEADS = 4
CIN = 256
HC = 64
HD = 512          # h width = 2 (s) * 4 (heads) * 64 (d)
B = 1024
NCORES = 8
NS = B // NCORES  # samples per core = 128
P = 128
G = 8             # samples per output group
NGRP = NS // G    # 16 groups per core
NEG = -9e15

CHS = 16                    # samples per xT chunk tile
NCH = NS // CHS             # 8 chunks
CHW = CHS * NLM             # 1568 cols per chunk
OVL = 32                    # overlap cols (next chunk head), xbar-tile mult

_CACHE = {}


def _build_nc():
    import concourse.mybir as mybir
    import concourse.tile as tile
    from concourse import bacc

    f16 = mybir.dt.float16
    f32 = mybir.dt.float32
    MUL = mybir.AluOpType.mult
    ADD = mybir.AluOpType.add

    nc = bacc.Bacc(None, target_bir_lowering=False)

    # x fp16, pre-transposed on host: [2, 128, NS*NLM] (c-half, c, row)
    xsp = nc.dram_tensor("xsp", [2, P, NS * NLM], f16, kind="ExternalInput")
    # packed consts: wall [p,(2,512)] ++ gmat [p,512] ++ dgv-f16 [p,4]
    CPW = 2 * HD + HEADS * P + HEADS
    cpak = nc.dram_tensor("cpak", [P, CPW], f16, kind="ExternalInput")
    brow = nc.dram_tensor("brow", [1, G * HD], f16, kind="ExternalInput")
    # node-major fp16 output [i, s, c]
    out = nc.dram_tensor("out", [NLM, NS, CIN], f16, kind="ExternalOutput")

    with tile.TileContext(nc) as tc:
        with (
            tc.tile_pool(name="const", bufs=1) as constp,
            tc.tile_pool(name="xt", bufs=1) as xtp,
            tc.tile_pool(name="hg", bufs=1) as hgp,
            tc.tile_pool(name="osb", bufs=10) as osbp,
            tc.tile_pool(name="phs", bufs=3, space="PSUM") as phsp,
            tc.tile_pool(name="pout", bufs=2, space="PSUM") as poutp,
        ):
            cp_sb = constp.tile([P, CPW], f16, tag="cpak")
            wall_sb = cp_sb[:, 0 : 2 * HD].rearrange("p (a f) -> p a f", a=2)
            gm_sb = cp_sb[:, 2 * HD : 2 * HD + HEADS * P]
            dgv_sb = cp_sb[:, 2 * HD + HEADS * P : CPW]

            # chunk 0 split in two 8-sample tiles for an earlier phase-1
            # start; chunks 1..7 are 16 samples each
            HCW = CHW // 2
            xt0 = [
                xtp.tile([P, 2, HCW + OVL], f16, tag=f"xt0{h}", name=f"xt0{h}")
                for h in range(2)
            ]
            xt = [
                xtp.tile([P, 2, CHW + OVL], f16, tag=f"xt{c}", name=f"xt{c}")
                for c in range(1, NCH)
            ]
            nc.vector.memset(xt[NCH - 2][:, :, CHW:], 0.0)

            hgt = [
                hgp.tile([P, G * HD], f16, tag=f"hg{i}", name=f"hg{i}")
                for i in range(2)
            ]

            # consts on the scalar HWDGE queue; x^T chunk loads are plain
            # DMAs (x pre-transposed host-side -> contiguous 3.2KB
            # descriptors), chunk 0 halves first, alternating queues
            nc.scalar.dma_start(cp_sb[:], cpak[:])
            # tiny brow loads first: phase-1 copies into hgt carry a WAW
            # dep on them, so they must not queue behind the bulk x loads
            nc.sync.dma_start(hgt[0][98:99, :], brow[:])
            nc.scalar.dma_start(hgt[1][98:99, :], brow[:])

            def xload(dst_ap, cc, r0, rw, eng):
                eng.dma_start(dst_ap, xsp[cc, :, r0 : r0 + rw])

            for h in range(2):
                for cc in range(2):
                    xload(
                        xt0[h][:, cc, :], cc, h * HCW, HCW + OVL,
                        nc.sync if cc == 0 else nc.scalar,
                    )
            for c in range(1, NCH):
                r0 = c * CHW
                rw = CHW + OVL if c < NCH - 1 else CHW
                for cc in range(2):
                    xload(
                        xt[c - 1][:, cc, 0:rw], cc, r0, rw,
                        nc.sync if cc == 0 else nc.scalar,
                    )

            hg3s = [h[:].rearrange("p (s f) -> p s f", s=G) for h in hgt]

            def emit_p2_head(gi, hd, osb3):
                """Phase 2 for one head of group gi: one k=99 matmul
                (A_off^T + bias row), then fused  out = dg (.) h0 + psum."""
                hg3 = hg3s[gi % 2]
                pouts = poutp.tile([P, G * HC], f32, tag="pout")
                po3 = pouts[:].rearrange("p (s f) -> p s f", s=G)
                nc.tensor.matmul(
                    po3,
                    gm_sb[0:99, hd * P : (hd + 1) * P],
                    hg3[0:99, :, 256 + hd * HC : 256 + (hd + 1) * HC],
                    start=True,
                    stop=True,
                )
                nc.vector.scalar_tensor_tensor(
                    out=osb3[:, :, hd * HC : (hd + 1) * HC],
                    in0=hg3[0:98, :, hd * HC : (hd + 1) * HC],
                    scalar=dgv_sb[0:98, hd : hd + 1],
                    in1=po3[0:98],
                    op0=MUL,
                    op1=ADD,
                )

            def store(gi, osb3, s0=0, s1=G):
                ov = out[:, gi * G + s0 : gi * G + s1, :]
                nc.gpsimd.dma_start(ov, osb3[0:98, s0:s1])

            osb_t = {}

            def emit_b_phase1(gi, prev):
                """Phase 1 for G samples of gi; phase 2 of group `prev`
                interleaved between the pairs."""
                hgrp = hgt[gi % 2]
                if prev is not None:
                    osb = osbp.tile([NLM, G * 256], f16, tag="osb")
                    osb3 = osb[:].rearrange("p (s c) -> p s c", s=G)
                    osb_t[prev] = osb3
                if gi < 2:
                    src, sb = xt0[gi], gi * G
                else:
                    src, sb = xt[gi // 2 - 1], (gi // 2) * CHS
                for pi in range(G // 2):
                    hps = phsp.tile([P, 2, HD], f32, tag="hps")
                    for a in range(2):
                        b = gi * G + pi * 2 + a
                        lb = b - sb
                        for cc in range(2):
                            nc.tensor.matmul(
                                hps[:, a, :],
                                src[:, cc, NLM * lb : NLM * lb + P],
                                wall_sb[:, cc, :],
                                start=(cc == 0),
                                stop=(cc == 1),
                            )
                    dst = hgrp[0:98, pi * 2 * HD : (pi + 1) * 2 * HD].rearrange(
                        "p (a f) -> p a f", a=2
                    )
                    # balance PSUM->SBUF copies: DVE takes one of four;
                    # for the final group split 2/2 for a shorter tail
                    on_dve = (pi == 0 and gi % 2 == 0) or (
                        gi == NGRP - 1 and pi % 2 == 1
                    )
                    if on_dve:
                        nc.vector.tensor_copy(dst, hps[0:98])
                    else:
                        nc.scalar.copy(out=dst, in_=hps[0:98])
                    if prev is not None:
                        if pi < 3:
                            emit_p2_head(prev, pi, osb_t[prev])
                        else:
                            emit_p2_head(prev, 3, osb_t[prev])
                            store(prev, osb_t[prev])

            def emit_p2_flush(gi):
                """Phase 2 for the final group: full-width ops, stores
                split in halves for a short kernel tail."""
                osb = osbp.tile([NLM, G * 256], f16, tag="osb")
                osb3 = osb[:].rearrange("p (s c) -> p s c", s=G)
                for hd in range(HEADS):
                    emit_p2_head(gi, hd, osb3)
                store(gi, osb3, 0, 4)
                store(gi, osb3, 4, 8)

            # ---- main emission ------------------------------------------
            # PE warmup: dummy matmuls on the const tile while waiting for
            # the first xT chunk, so real matmuls start at full p-state
            for w in range(4):
                hps = phsp.tile([P, 2, HD], f32, tag="hps")
                nc.tensor.matmul(
                    hps[:, 0, :],
                    cp_sb[:, 0:P],
                    wall_sb[:, 0, :],
                    start=True,
                    stop=True,
                )
            prev = None
            for gi in range(NGRP):
                emit_b_phase1(gi, prev)
                prev = gi
            emit_p2_flush(prev)

    nc.compile()
    return nc


def _host_prep(W, e, bias, rows, cols):
    """Precompute fp16 device constants from the small parameter tensors."""
    W = np.asarray(W, np.float32)
    e = np.asarray(e, np.float32)
    bias = np.asarray(bias, np.float32)
    rows = np.asarray(rows, np.int64)
    cols = np.asarray(cols, np.int64)

    logits = np.full((HEADS, NLM, NLM), NEG, np.float64)
    logits[:, rows, cols] = e.astype(np.float64)
    m = logits.max(axis=-1, keepdims=True)
    p = np.exp(logits - m)
    A = p / p.sum(axis=-1, keepdims=True)            # [H, N, N]
    dg = np.einsum("hii->hi", A).copy()              # [H, N]
    A_off = A.copy()
    np.einsum("hii->hi", A_off)[:] = 0.0

    # Wall: [c, (s, h, d)] -> chunked [128, 2, 512]
    wr = W.transpose(2, 1, 0, 3).reshape(CIN, 2 * HEADS * HC)   # [c, shd]
    wall = np.ascontiguousarray(
        wr.reshape(2, P, 2 * HEADS * HC).transpose(1, 0, 2)
    ).astype(np.float16)

    # graph matrices: [j, (head, i)]; row 98 = all-ones bias row
    gm = np.zeros((P, HEADS, P), np.float32)
    for h in range(HEADS):
        gm[:NLM, h, :NLM] = A_off[h].T
        gm[NLM, h, :NLM] = 1.0
    gmat = gm.reshape(P, HEADS * P).astype(np.float16)

    dgvt = np.zeros((P, HEADS), np.float16)
    dgvt[:NLM] = dg.T                                           # [98, 4]

    # packed consts: wall(1024) ++ gmat(512) ++ dgv(4) per partition
    cpak = np.ascontiguousarray(
        np.concatenate(
            [wall.reshape(P, 2 * HD), gmat, dgvt], axis=1
        )
    ).astype(np.float16)

    # bias row pattern for hgrp row 98: [s, (part, h, d)], part-1 = bias
    br = np.zeros((G, 2, HEADS * HC), np.float32)
    br[:, 1, :] = bias.reshape(HEADS * HC)
    brow = np.ascontiguousarray(br.reshape(1, G * HD)).astype(np.float16)

    return {"cpak": cpak, "brow": brow}


def kernel(x, W, e, bias, rows, cols):
    from concourse.bass_utils import run_bass_kernel_spmd

    if "nc" not in _CACHE:
        _CACHE["nc"] = _build_nc()
    nc = _CACHE["nc"]

    consts = _host_prep(W, e, bias, rows, cols)
    # [B*NLM, 256] fp16 -> per-core pre-transposed [2, 128, NS*NLM]
    x16 = np.asarray(x, np.float32).reshape(B * NLM, CIN).astype(np.float16)
    xsp_all = np.ascontiguousarray(
        x16.reshape(NCORES, NS * NLM, 2, P).transpose(0, 2, 3, 1)
    )

    in_maps = []
    for ci in range(NCORES):
        in_maps.append({"xsp": xsp_all[ci], **consts})

    res = run_bass_kernel_spmd(
        nc,
        in_maps,
        core_ids=list(range(NCORES)),
        trace=bool(int(os.environ.get("KERNEL_TRACE", "0"))),
    )
    _CACHE["last_results"] = res

    # device out is [98, 128, 256] f16 node-major; back to [NS, 98, 256]
    out = np.concatenate(
        [
            r["out"].reshape(NLM, NS, HEADS * HC).transpose(1, 0, 2)
            for r in res.results
        ],
        axis=0,
    ).astype(np.float32)
    return out.reshape(B, NLM, HEADS * HC)


# revision 31
# speedup vs baseline: 1.0874x; 1.0874x over previous
"""MultiHeadSemGConv Trainium2 kernel.

Computes, for x:[B,N,CIN], W:[H,2,CIN,HC], e:[H,N*K], bias:[H,HC],
rows/cols:[N*K] (int32 edge list):

    h = einsum('bnc,hscd->shbnd', x, W)             # two projections per head
    A = softmax(scatter(e at (rows,cols), NEG))     # [H,N,N]
    out[h,b] = diag(A)*h0 + (A - diag)@h1 + bias    # -> [B,N,H*HC]

Strategy: pure data-parallel over batch across 8 NeuronCores.  The tiny
[H,98,98] adjacency softmax is precomputed on host; the heavy lifting
(x projection + graph mixing over 100MB of activations) runs on device:

  per core (128 samples; x pre-cast to fp16 AND pre-transposed to
  [2, 128, 12544] host-side, so the device never transposes):
    - x^T chunk tiles [c(2x128), 1600] fp16 (16 samples + 32-col
      overlap) loaded by plain DMAs with contiguous 3.2KB descriptors.
    - phase 1, per sample b: h[128,512] = xT[:, 98b:98b+128].T @ Wall
      (2 accumulating fp16 matmuls, f32 PSUM), 2 samples per PSUM tile;
      rows [:98] copied (Act/DVE) into one of two persistent h tiles
      whose row 98 holds the bias pattern.  The 30-row overlap between
      consecutive samples keeps every lhsT at m=128.
    - phase 2, per 8-sample group, per head: ONE matmul with the
      host-built A_off^T (contract k=99: 98 nodes + bias row), then a
      fused DVE op  out = dg (.) h0 + psum  adds the diagonal part.
      Phase 2 of group g is interleaved into phase 1 of group g+1.
    - DMA out fp16 in node-major layout [98, 128, 256] (4KB
      descriptors); host transposes back to [B,N,256] f32.
"""

import os
import sys

import numpy as np

try:
    import concourse.bass as bass  # noqa: F401
except Exception:  # pragma: no cover - fresh grading dir fallback
    for p in ("/opt/trn_rl_repo", "/root/.axon_site/_ro/trn_rl_repo"):
        if os.path.isdir(p) and p not in sys.path:
            sys.path.insert(0, p)
    import concourse.bass as bass  # noqa: F401

# ---------------------------------------------------------------- constants
NLM = 98          # landmarks (graph nodes)
HEADS = 4
CIN = 256
HC = 64
HD = 512          # h width = 2 (s) * 4 (heads) * 64 (d)
B = 1024
NCORES = 8
NS = B // NCORES  # samples per core = 128
P = 128
G = 8             # samples per output group
NGRP = NS // G    # 16 groups per core
NEG = -9e15

CHS = 16                    # samples per xT chunk tile
NCH = NS // CHS             # 8 chunks
CHW = CHS * NLM             # 1568 cols per chunk
OVL = 32                    # overlap cols (next chunk head), xbar-tile mult

_CACHE = {}


def _build_nc():
    import concourse.mybir as mybir
    import concourse.tile as tile
    from concourse import bacc

    f16 = mybir.dt.float16
    f32 = mybir.dt.float32
    MUL = mybir.AluOpType.mult
    ADD = mybir.AluOpType.add

    nc = bacc.Bacc(None, target_bir_lowering=False)

    # x fp16, pre-transposed on host: [2, 128, NS*NLM] (c-half, c, row)
    xsp = nc.dram_tensor("xsp", [2, P, NS * NLM], f16, kind="ExternalInput")
    # packed consts: wall [p,(2,512)] ++ gmat [p,512] ++ dgv-f16 [p,4]
    CPW = 2 * HD + HEADS * P + HEADS
    cpak = nc.dram_tensor("cpak", [P, CPW], f16, kind="ExternalInput")
    brow = nc.dram_tensor("brow", [1, G * HD], f16, kind="ExternalInput")
    # node-major fp16 output [i, s, c]
    out = nc.dram_tensor("out", [NLM, NS, CIN], f16, kind="ExternalOutput")

    with tile.TileContext(nc) as tc:
        with (
            tc.tile_pool(name="const", bufs=1) as constp,
            tc.tile_pool(name="xt", bufs=1) as xtp,
            tc.tile_pool(name="hg", bufs=1) as hgp,
            tc.tile_pool(name="osb", bufs=10) as osbp,
            tc.tile_pool(name="phs", bufs=3, space="PSUM") as phsp,
            tc.tile_pool(name="pout", bufs=2, space="PSUM") as poutp,
        ):
            cp_sb = constp.tile([P, CPW], f16, tag="cpak")
            wall_sb = cp_sb[:, 0 : 2 * HD].rearrange("p (a f) -> p a f", a=2)
            gm_sb = cp_sb[:, 2 * HD : 2 * HD + HEADS * P]
            dgv_sb = cp_sb[:, 2 * HD + HEADS * P : CPW]

            # chunk 0 split in two 8-sample tiles for an earlier phase-1
            # start; chunks 1..7 are 16 samples each
            HCW = CHW // 2
            xt0 = [
                xtp.tile([P, 2, HCW + OVL], f16, tag=f"xt0{h}", name=f"xt0{h}")
                for h in range(2)
            ]
            xt = [
                xtp.tile([P, 2, CHW + OVL], f16, tag=f"xt{c}", name=f"xt{c}")
                for c in range(1, NCH)
            ]
            nc.vector.memset(xt[NCH - 2][:, :, CHW:], 0.0)

            hgt = [
                hgp.tile([P, G * HD], f16, tag=f"hg{i}", name=f"hg{i}")
                for i in range(2)
            ]

            # consts on the scalar HWDGE queue; x^T chunk loads are plain
            # DMAs (x pre-transposed host-side -> contiguous 3.2KB
            # descriptors), chunk 0 halves first, alternating queues
            nc.scalar.dma_start(cp_sb[:], cpak[:])

            def xload(dst_ap, cc, r0, rw, eng):
                eng.dma_start(dst_ap, xsp[cc, :, r0 : r0 + rw])

            for h in range(2):
                for cc in range(2):
                    xload(
                        xt0[h][:, cc, :], cc, h * HCW, HCW + OVL,
                        nc.sync if cc == 0 else nc.scalar,
                    )
            nc.scalar.dma_start(hgt[0][98:99, :], brow[:])
            nc.scalar.dma_start(hgt[1][98:99, :], brow[:])
            # bulk loads all on the sync queue: the Act sequencer pays
            # 667ns per DMA trigger and must stay free for phase-1 copies
            for c in range(1, NCH):
                r0 = c * CHW
                rw = CHW + OVL if c < NCH - 1 else CHW
                for cc in range(2):
                    xload(xt[c - 1][:, cc, 0:rw], cc, r0, rw, nc.sync)

            hg3s = [h[:].rearrange("p (s f) -> p s f", s=G) for h in hgt]

            def emit_p2_head(gi, hd, osb3):
                """Phase 2 for one head of group gi: one k=99 matmul
                (A_off^T + bias row), then fused  out = dg (.) h0 + psum."""
                hg3 = hg3s[gi % 2]
                pouts = poutp.tile([P, G * HC], f32, tag="pout")
                po3 = pouts[:].rearrange("p (s f) -> p s f", s=G)
                nc.tensor.matmul(
                    po3,
                    gm_sb[0:99, hd * P : (hd + 1) * P],
                    hg3[0:99, :, 256 + hd * HC : 256 + (hd + 1) * HC],
                    start=True,
                    stop=True,
                )
                nc.vector.scalar_tensor_tensor(
                    out=osb3[:, :, hd * HC : (hd + 1) * HC],
                    in0=hg3[0:98, :, hd * HC : (hd + 1) * HC],
                    scalar=dgv_sb[0:98, hd : hd + 1],
                    in1=po3[0:98],
                    op0=MUL,
                    op1=ADD,
                )

            def store(gi, osb3, s0=0, s1=G):
                ov = out[:, gi * G + s0 : gi * G + s1, :]
                nc.gpsimd.dma_start(ov, osb3[0:98, s0:s1])

            osb_t = {}

            def emit_b_phase1(gi, prev):
                """Phase 1 for G samples of gi; phase 2 of group `prev`
                interleaved between the pairs."""
                hgrp = hgt[gi % 2]
                if prev is not None:
                    osb = osbp.tile([NLM, G * 256], f16, tag="osb")
                    osb3 = osb[:].rearrange("p (s c) -> p s c", s=G)
                    osb_t[prev] = osb3
                if gi < 2:
                    src, sb = xt0[gi], gi * G
                else:
                    src, sb = xt[gi // 2 - 1], (gi // 2) * CHS
                for pi in range(G // 2):
                    hps = phsp.tile([P, 2, HD], f32, tag="hps")
                    for a in range(2):
                        b = gi * G + pi * 2 + a
                        lb = b - sb
                        for cc in range(2):
                            nc.tensor.matmul(
                                hps[:, a, :],
                                src[:, cc, NLM * lb : NLM * lb + P],
                                wall_sb[:, cc, :],
                                start=(cc == 0),
                                stop=(cc == 1),
                            )
                    dst = hgrp[0:98, pi * 2 * HD : (pi + 1) * 2 * HD].rearrange(
                        "p (a f) -> p a f", a=2
                    )
                    # balance PSUM->SBUF copies: DVE takes one of four;
                    # for the final group split 2/2 for a shorter tail
                    on_dve = (pi == 0 and gi % 2 == 0) or (
                        gi == NGRP - 1 and pi % 2 == 1
                    )
                    if on_dve:
                        nc.vector.tensor_copy(dst, hps[0:98])
                    else:
                        nc.scalar.copy(out=dst, in_=hps[0:98])
                    if prev is not None:
                        if pi < 3:
                            emit_p2_head(prev, pi, osb_t[prev])
                        else:
                            emit_p2_head(prev, 3, osb_t[prev])
                            store(prev, osb_t[prev])

            def emit_p2_flush(gi):
                """Phase 2 for the final group: full-width ops, stores
                split in halves for a short kernel tail."""
                osb = osbp.tile([NLM, G * 256], f16, tag="osb")
                osb3 = osb[:].rearrange("p (s c) -> p s c", s=G)
                for hd in range(HEADS):
                    emit_p2_head(gi, hd, osb3)
                store(gi, osb3, 0, 4)
                store(gi, osb3, 4, 8)

            # ---- main emission ------------------------------------------
            # PE warmup: dummy matmuls on the const tile while waiting for
            # the first xT chunk, so real matmuls start at full p-state
            for w in range(4):
                hps = phsp.tile([P, 2, HD], f32, tag="hps")
                nc.tensor.matmul(
                    hps[:, 0, :],
                    cp_sb[:, 0:P],
                    wall_sb[:, 0, :],
                    start=True,
                    stop=True,
                )
            prev = None
            for gi in range(NGRP):
                emit_b_phase1(gi, prev)
                prev = gi
            emit_p2_flush(prev)

    nc.compile()
    return nc


def _host_prep(W, e, bias, rows, cols):
    """Precompute fp16 device constants from the small parameter tensors."""
    W = np.asarray(W, np.float32)
    e = np.asarray(e, np.float32)
    bias = np.asarray(bias, np.float32)
    rows = np.asarray(rows, np.int64)
    cols = np.asarray(cols, np.int64)

    logits = np.full((HEADS, NLM, NLM), NEG, np.float64)
    logits[:, rows, cols] = e.astype(np.float64)
    m = logits.max(axis=-1, keepdims=True)
    p = np.exp(logits - m)
    A = p / p.sum(axis=-1, keepdims=True)            # [H, N, N]
    dg = np.einsum("hii->hi", A).copy()              # [H, N]
    A_off = A.copy()
    np.einsum("hii->hi", A_off)[:] = 0.0

    # Wall: [c, (s, h, d)] -> chunked [128, 2, 512]
    wr = W.transpose(2, 1, 0, 3).reshape(CIN, 2 * HEADS * HC)   # [c, shd]
    wall = np.ascontiguousarray(
        wr.reshape(2, P, 2 * HEADS * HC).transpose(1, 0, 2)
    ).astype(np.float16)

    # graph matrices: [j, (head, i)]; row 98 = all-ones bias row
    gm = np.zeros((P, HEADS, P), np.float32)
    for h in range(HEADS):
        gm[:NLM, h, :NLM] = A_off[h].T
        gm[NLM, h, :NLM] = 1.0
    gmat = gm.reshape(P, HEADS * P).astype(np.float16)

    dgvt = np.zeros((P, HEADS), np.float16)
    dgvt[:NLM] = dg.T                                           # [98, 4]

    # packed consts: wall(1024) ++ gmat(512) ++ dgv(4) per partition
    cpak = np.ascontiguousarray(
        np.concatenate(
            [wall.reshape(P, 2 * HD), gmat, dgvt], axis=1
        )
    ).astype(np.float16)

    # bias row pattern for hgrp row 98: [s, (part, h, d)], part-1 = bias
    br = np.zeros((G, 2, HEADS * HC), np.float32)
    br[:, 1, :] = bias.reshape(HEADS * HC)
    brow = np.ascontiguousarray(br.reshape(1, G * HD)).astype(np.float16)

    return {"cpak": cpak, "brow": brow}


def kernel(x, W, e, bias, rows, cols):
    from concourse.bass_utils import run_bass_kernel_spmd

    if "nc" not in _CACHE:
        _CACHE["nc"] = _build_nc()
    nc = _CACHE["nc"]

    consts = _host_prep(W, e, bias, rows, cols)
    # [B*NLM, 256] fp16 -> per-core pre-transposed [2, 128, NS*NLM]
    x16 = np.asarray(x, np.float32).reshape(B * NLM, CIN).astype(np.float16)
    xsp_all = np.ascontiguousarray(
        x16.reshape(NCORES, NS * NLM, 2, P).transpose(0, 2, 3, 1)
    )

    in_maps = []
    for ci in range(NCORES):
        in_maps.append({"xsp": xsp_all[ci], **consts})

    res = run_bass_kernel_spmd(
        nc,
        in_maps,
        core_ids=list(range(NCORES)),
        trace=bool(int(os.environ.get("KERNEL_TRACE", "0"))),
    )
    _CACHE["last_results"] = res

    # device out is [98, 128, 256] f16 node-major; back to [NS, 98, 256]
    out = np.concatenate(
        [
            r["out"].reshape(NLM, NS, HEADS * HC).transpose(1, 0, 2)
            for r in res.results
        ],
        axis=0,
    ).astype(np.float32)
    return out.reshape(B, NLM, HEADS * HC)
